# revision 9
# baseline (speedup 1.0000x reference)
"""Equiformer DTP-by-head message-passing kernel for Trainium2 (Bass/Tile).

Full inputs in, full outputs out; internally shards the node dim i across
8 NeuronCores (pure edge parallelism, no cross-core comm).

Math (per head h, edge e=(i,j), channel m):
  ss = w_ss*s ; t = w_sv*s ; vs = w_vs*v ; vv = w_vv*(v.r)
  s_out[h,e,:] = w_s[h][:, :16] @ ss + w_s[h][:, 16:] @ vv + b_s[h]
  v_out[h,e,:,k] = r[e,k] * (w_v[h][:, :16] @ t) + w_v[h][:, 16:] @ vs[...,k]

Device-side layout: edges on partitions (128 per sub-chunk), (h,m)=128 on
the free dim. Products are built with DVE, PE-transposed to channel-major,
then one block-diagonal K=128 matmul per term computes all 8 heads at once
with the output back in edge-major layout (contiguous stores).

The op is memory-bound, so host packs w/s/v into one edge-major bf16 tensor
[E, 1024] and the device writes one bf16 [E, 768] output tensor (host
upcasts); r stays f32 in a small side tensor. Every big DMA moves >=1.5KB
contiguous runs at full HBM rate with one in + one out launch per 512 edges.
bf16 keeps elementwise relative error bounded by 2^-9 while halving traffic.
"""

import functools

import numpy as np
import ml_dtypes

H = 8
I_FULL = 4096
J = 32
M = 16
CS = 48  # NC_S_OUT
CV = 16  # NC_V_OUT
NCORES = 8
ILOC = I_FULL // NCORES  # 512
E_LOC = ILOC * J  # 16384 edges per core
P = 128
Q = 4  # sub-chunks per macro chunk
NMACRO = E_LOC // (P * Q)  # 32
HM = H * M  # 128
HCS = H * CS  # 384
HMK = HM * 3  # 384

# packed bf16 input layout per edge
W_OFF = 0          # 512 words: weights (c,h,m)
S_OFF = 512        # 128 words: s (h,m)
V_OFF = 640        # 384 words: v (h,m,k)
IN_W = 1024
R_W = 4            # separate f32 side tensor: r (3) + pad
# packed bf16 output layout per edge
SO_OFF = 0         # 384 words: s_out (h,c)
VO_OFF = 384       # 384 words: v_out (h,m,k)
OUT_W = 768


@functools.lru_cache(maxsize=2)
def _build(with_bias: bool):
    import concourse.bacc as bacc
    import concourse.tile as tile
    from concourse import mybir
    from contextlib import ExitStack

    f32 = mybir.dt.float32
    bf16 = mybir.dt.bfloat16
    mult = mybir.AluOpType.mult
    add = mybir.AluOpType.add

    nc = bacc.Bacc(trn_type="TRN2", num_devices=NCORES)

    in_d = nc.dram_tensor("pack_in", [E_LOC, IN_W], bf16, kind="ExternalInput")
    r_d = nc.dram_tensor("pack_r", [E_LOC, R_W], f32, kind="ExternalInput")
    ws1_d = nc.dram_tensor("Ws1", [HM, HCS], bf16, kind="ExternalInput")
    ws2_d = nc.dram_tensor("Ws2", [HM, HCS], bf16, kind="ExternalInput")
    wv1_d = nc.dram_tensor("Wv1", [HM, HM], bf16, kind="ExternalInput")
    wv2_d = nc.dram_tensor("Wv2", [HM, HM], bf16, kind="ExternalInput")
    bias_d = nc.dram_tensor("bias_row", [1, HCS], bf16, kind="ExternalInput")
    ones_d = nc.dram_tensor("ones_row", [1, P], bf16, kind="ExternalInput")
    ident_d = nc.dram_tensor("ident", [P, P], bf16, kind="ExternalInput")
    out_d = nc.dram_tensor("pack_out", [E_LOC, OUT_W], bf16, kind="ExternalOutput")

    # [macro, partition(=edge within sub-chunk), sub-chunk, word]
    in_view = in_d.ap().rearrange("(n q p) w -> n p q w", q=Q, p=P)
    r_view = r_d.ap().rearrange("(n q p) w -> n p q w", q=Q, p=P)
    out_view = out_d.ap().rearrange("(n q p) w -> n p q w", q=Q, p=P)

    with tile.TileContext(nc) as tc, ExitStack() as ctx:
        consts = ctx.enter_context(tc.tile_pool(name="consts", bufs=1))
        io = ctx.enter_context(tc.tile_pool(name="io", bufs=3))
        prod = ctx.enter_context(tc.tile_pool(name="prod", bufs=3))
        tsb = ctx.enter_context(tc.tile_pool(name="tsb", bufs=3))
        outs = ctx.enter_context(tc.tile_pool(name="outs", bufs=3))
        tp = ctx.enter_context(tc.tile_pool(name="tp", bufs=3, space="PSUM"))
        pss = ctx.enter_context(tc.tile_pool(name="pss", bufs=3, space="PSUM"))
        psa = ctx.enter_context(tc.tile_pool(name="psa", bufs=1, space="PSUM"))
        psv = ctx.enter_context(tc.tile_pool(name="psv", bufs=1, space="PSUM"))

        ws1_t = consts.tile([HM, HCS], bf16)
        ws2_t = consts.tile([HM, HCS], bf16)
        wv1_t = consts.tile([HM, HM], bf16)
        wv2_t = consts.tile([HM, HM], bf16)
        bias_t = consts.tile([1, HCS], bf16)
        ones_t = consts.tile([1, P], bf16)
        ident_t = consts.tile([P, P], bf16)
        nc.sync.dma_start(out=ws1_t[:], in_=ws1_d[:])
        nc.sync.dma_start(out=ws2_t[:], in_=ws2_d[:])
        nc.sync.dma_start(out=wv1_t[:], in_=wv1_d[:])
        nc.sync.dma_start(out=wv2_t[:], in_=wv2_d[:])
        nc.sync.dma_start(out=bias_t[:], in_=bias_d[:])
        nc.sync.dma_start(out=ones_t[:], in_=ones_d[:])
        nc.sync.dma_start(out=ident_t[:], in_=ident_d[:])

        for n in range(NMACRO):
            in_t = io.tile([P, Q, IN_W], bf16)
            r_t = io.tile([P, Q, R_W], f32)
            nc.sync.dma_start(out=in_t[:], in_=in_view[n])
            nc.sync.dma_start(out=r_t[:], in_=r_view[n])
            out_t = outs.tile([P, Q, OUT_W], bf16)

            for q in range(Q):
                blk = in_t[:, q]
                w_ss = blk[:, 0 * HM : 1 * HM]
                w_sv = blk[:, 1 * HM : 2 * HM]
                w_vs = blk[:, 2 * HM : 3 * HM].rearrange("p (h m) -> p h m", h=H)
                w_vv = blk[:, 3 * HM : 4 * HM].rearrange("p (h m) -> p h m", h=H)
                s_flat = blk[:, S_OFF : S_OFF + HM]
                v_b = blk[:, V_OFF : V_OFF + HMK].rearrange(
                    "p (h m k) -> p h m k", h=H, k=3
                )
                r_b = r_t[:, q]

                ss_t = prod.tile([P, HM], bf16, tag="ss")
                t_t = prod.tile([P, HM], bf16, tag="t")
                u_t = prod.tile([P, 3, H, M], bf16, tag="u")
                d1_t = prod.tile([P, H, M], f32, tag="d1")
                d2_t = prod.tile([P, H, M], f32, tag="d2")
                d3_t = prod.tile([P, H, M], f32, tag="d3")
                vv_t = prod.tile([P, H, M], bf16, tag="vv")

                nc.vector.tensor_tensor(out=ss_t[:], in0=w_ss, in1=s_flat, op=mult)
                nc.vector.tensor_tensor(out=t_t[:], in0=w_sv, in1=s_flat, op=mult)
                for k in range(3):
                    nc.vector.tensor_tensor(
                        out=u_t[:, k], in0=w_vs, in1=v_b[:, :, :, k], op=mult
                    )
                nc.vector.tensor_scalar_mul(
                    out=d1_t[:], in0=v_b[:, :, :, 0], scalar1=r_b[:, 0:1]
                )
                nc.vector.scalar_tensor_tensor(
                    out=d2_t[:], in0=v_b[:, :, :, 1], scalar=r_b[:, 1:2],
                    in1=d1_t[:], op0=mult, op1=add,
                )
                nc.vector.scalar_tensor_tensor(
                    out=d3_t[:], in0=v_b[:, :, :, 2], scalar=r_b[:, 2:3],
                    in1=d2_t[:], op0=mult, op1=add,
                )
                nc.vector.tensor_tensor(out=vv_t[:], in0=d3_t[:], in1=w_vv, op=mult)

                # PE transposes to channel-major + PSUM->SBUF copies
                ssT = tsb.tile([HM, P], bf16, tag="ssT")
                tT = tsb.tile([HM, P], bf16, tag="tT")
                vvT = tsb.tile([HM, P], bf16, tag="vvT")
                uT = [
                    tsb.tile([HM, P], bf16, tag=f"uT{k}", name=f"uT{k}")
                    for k in range(3)
                ]
                for src, dst in (
                    (ss_t[:], ssT),
                    (t_t[:], tT),
                    (vv_t[:].rearrange("p h m -> p (h m)"), vvT),
                    (u_t[:, 0].rearrange("p h m -> p (h m)"), uT[0]),
                    (u_t[:, 1].rearrange("p h m -> p (h m)"), uT[1]),
                    (u_t[:, 2].rearrange("p h m -> p (h m)"), uT[2]),
                ):
                    p_t = tp.tile([P, P], bf16, tag="tp")
                    nc.tensor.transpose(p_t[:], src, ident_t[:])
                    nc.scalar.copy(out=dst[:], in_=p_t[:])

                # Block-diagonal matmuls, K=128=(h,m); outputs edge-major.
                pss_t = pss.tile([P, HCS], f32, tag="pss")
                nc.tensor.matmul(
                    pss_t[:], ssT[:], ws1_t[:], start=True, stop=False
                )
                nc.tensor.matmul(
                    pss_t[:], vvT[:], ws2_t[:],
                    start=False, stop=not with_bias,
                )
                if with_bias:
                    nc.tensor.matmul(
                        pss_t[:], ones_t[:], bias_t[:], start=False, stop=True
                    )
                psa_t = psa.tile([P, H, CV], f32, tag="psa")
                nc.tensor.matmul(
                    psa_t[:].rearrange("p h m -> p (h m)"), tT[:], wv1_t[:],
                    start=True, stop=True,
                )
                a_sb = tsb.tile([P, H, CV], f32, tag="a_sb")
                nc.scalar.copy(out=a_sb[:], in_=psa_t[:])
                psv_t = psv.tile([P, 3, H, CV], f32, tag="psv")
                for k in range(3):
                    nc.tensor.matmul(
                        psv_t[:, k].rearrange("p h m -> p (h m)"), uT[k][:],
                        wv2_t[:], start=True, stop=True,
                    )

                nc.scalar.copy(
                    out=out_t[:, q, SO_OFF : SO_OFF + HCS], in_=pss_t[:]
                )
                vo_b = out_t[:, q, VO_OFF : VO_OFF + HMK].rearrange(
                    "p (h m k) -> p h m k", h=H, k=3
                )
                for k in range(3):
                    nc.vector.scalar_tensor_tensor(
                        out=vo_b[:, :, :, k], in0=a_sb[:],
                        scalar=r_b[:, k : k + 1],
                        in1=psv_t[:, k], op0=mult, op1=add,
                    )
            nc.sync.dma_start(out=out_view[n], in_=out_t[:])

    nc.compile()
    return nc


def _prep_weights(w_s, b_s, w_v):
    bf16 = ml_dtypes.bfloat16
    ws1 = np.zeros((HM, HCS), np.float32)
    ws2 = np.zeros((HM, HCS), np.float32)
    wv1 = np.zeros((HM, HM), np.float32)
    wv2 = np.zeros((HM, HM), np.float32)
    for h in range(H):
        ws1[h * M : (h + 1) * M, h * CS : (h + 1) * CS] = w_s[h, :, 0:M].T
        ws2[h * M : (h + 1) * M, h * CS : (h + 1) * CS] = w_s[h, :, M : 2 * M].T
        wv1[h * M : (h + 1) * M, h * CV : (h + 1) * CV] = w_v[h, :, 0:M].T
        wv2[h * M : (h + 1) * M, h * CV : (h + 1) * CV] = w_v[h, :, M : 2 * M].T
    bias = np.asarray(b_s, np.float32).reshape(1, HCS)
    ones = np.ones((1, P), np.float32)
    ident = np.eye(P, dtype=np.float32)
    return (
        ws1.astype(bf16), ws2.astype(bf16), wv1.astype(bf16), wv2.astype(bf16),
        bias.astype(bf16), ones.astype(bf16), ident.astype(bf16),
    )


def _pack_core(s, v, w, lo, hi):
    """Pack one core's slice into edge-major [E_LOC, IN_W] bf16."""
    bf16 = ml_dtypes.bfloat16
    pk = np.empty((E_LOC, IN_W), bf16)
    pk[:, W_OFF : W_OFF + 512] = w[lo:hi].reshape(E_LOC, 512)
    pk[:, S_OFF : S_OFF + HM] = (
        s[:, lo:hi].transpose(1, 2, 0, 3).reshape(E_LOC, HM)
    )
    pk[:, V_OFF : V_OFF + HMK] = (
        v[:, lo:hi].transpose(1, 2, 0, 3, 4).reshape(E_LOC, HMK)
    )
    return pk


def kernel(s, v, r_ij_vec, weights, w_s, b_s, w_v):
    from concourse.bass_utils import run_bass_kernel_spmd

    s = np.ascontiguousarray(np.asarray(s, np.float32))
    v = np.ascontiguousarray(np.asarray(v, np.float32))
    r = np.ascontiguousarray(np.asarray(r_ij_vec, np.float32))
    w = np.ascontiguousarray(np.asarray(weights, np.float32))
    b_s = np.asarray(b_s, np.float32)
    with_bias = bool(np.any(b_s != 0.0))
    nc = _build(with_bias)
    ws1, ws2, wv1, wv2, bias, ones, ident = _prep_weights(
        np.asarray(w_s, np.float32), b_s, np.asarray(w_v, np.float32)
    )

    in_maps = []
    for c in range(NCORES):
        lo, hi = c * ILOC, (c + 1) * ILOC
        pr = np.zeros((E_LOC, R_W), np.float32)
        pr[:, 0:3] = r[lo:hi].reshape(E_LOC, 3)
        in_maps.append({
            "pack_in": _pack_core(s, v, w, lo, hi),
            "pack_r": pr,
            "Ws1": ws1, "Ws2": ws2, "Wv1": wv1, "Wv2": wv2,
            "bias_row": bias, "ones_row": ones, "ident": ident,
        })

    res = run_bass_kernel_spmd(nc, in_maps, core_ids=list(range(NCORES)))

    s_parts, v_parts = [], []
    for c in range(NCORES):
        out = res.results[c]["pack_out"].astype(np.float32)
        s_parts.append(
            out[:, SO_OFF : SO_OFF + HCS]
            .reshape(ILOC, J, H, CS).transpose(2, 0, 1, 3)
        )
        v_parts.append(
            out[:, VO_OFF : VO_OFF + HMK]
            .reshape(ILOC, J, H, M, 3).transpose(2, 0, 1, 3, 4)
        )
    s_out = np.concatenate(s_parts, axis=1)
    v_out = np.concatenate(v_parts, axis=1)
    return s_out, v_out


# revision 12
# speedup vs baseline: 1.1603x; 1.1603x over previous
"""Equiformer DTP-by-head message-passing kernel for Trainium2 (Bass/Tile).

Full inputs in, full outputs out; internally shards the node dim i across
8 NeuronCores (pure edge parallelism, no cross-core comm).

Math (per head h, edge e=(i,j), channel m):
  ss = w_ss*s ; t = w_sv*s ; vs = w_vs*v ; vv = w_vv*(v.r)
  s_out[h,e,:] = w_s[h][:, :16] @ ss + w_s[h][:, 16:] @ vv + b_s[h]
  v_out[h,e,:,k] = r[e,k] * (w_v[h][:, :16] @ t) + w_v[h][:, 16:] @ vs[...,k]

The op is memory-bound; the kernel is organized so HBM traffic is minimal
and every DMA runs at full rate:
  * Host packs w/s/v CHANNEL-major ([hm=128 partitions, edges free], bf16)
    so the per-head contractions are K=128 block-diagonal matmuls with NO
    on-chip transposes at all: out = data_slice.T @ W lands edge-major.
  * r is broadcast across partitions with an exact K=1 ones.T@r_row matmul
    (float32r: full rate, keeps r at fp32 precision) for the v.r dot.
  * Outputs are written bf16 edge-major packed (elementwise relative error
    of bf16 is bounded by 2^-9) and upcast/unpacked on the host.
One input DMA + one output DMA (plus two tiny r DMAs) per 512 edges.
"""

import functools

import numpy as np
import ml_dtypes

H = 8
I_FULL = 4096
J = 32
M = 16
CS = 48  # NC_S_OUT
CV = 16  # NC_V_OUT
NCORES = 8
ILOC = I_FULL // NCORES  # 512
E_LOC = ILOC * J  # 16384 edges per core
P = 128
Q = 4  # sub-chunks (of 128 edges) per macro chunk
EM = P * Q  # 512 edges per macro
NMACRO = E_LOC // EM  # 32
HM = H * M  # 128
HCS = H * CS  # 384
HMK = HM * 3  # 384

# channel-major packed input: [hm=128, NMACRO, 8, EM] bf16
# rows: w_ss, w_sv, w_vs, w_vv, s, v_k0, v_k1, v_k2
ROW_WSS, ROW_WSV, ROW_WVS, ROW_WVV, ROW_S, ROW_V0 = 0, 1, 2, 3, 4, 5
# packed bf16 output layout per edge
SO_OFF = 0         # 384 words: s_out (h,c)
VO_OFF = 384       # 384 words: v_out (h,m,k)
OUT_W = 768
R_W = 4            # edge-major f32 r (3) + pad


@functools.lru_cache(maxsize=2)
def _build(with_bias: bool):
    import concourse.bacc as bacc
    import concourse.tile as tile
    from concourse import mybir
    from contextlib import ExitStack

    f32 = mybir.dt.float32
    f32r = mybir.dt.float32r
    bf16 = mybir.dt.bfloat16
    mult = mybir.AluOpType.mult
    add = mybir.AluOpType.add

    nc = bacc.Bacc(trn_type="TRN2", num_devices=NCORES)

    in_d = nc.dram_tensor("pack_cm", [P, NMACRO, 8, EM], bf16, kind="ExternalInput")
    rcm_d = nc.dram_tensor("r_cm", [NMACRO, 3, EM], f32r, kind="ExternalInput")
    r32_d = nc.dram_tensor("r32", [E_LOC, R_W], f32, kind="ExternalInput")
    ws1_d = nc.dram_tensor("Ws1", [HM, HCS], bf16, kind="ExternalInput")
    ws2_d = nc.dram_tensor("Ws2", [HM, HCS], bf16, kind="ExternalInput")
    wv1_d = nc.dram_tensor("Wv1", [HM, HM], bf16, kind="ExternalInput")
    wv2_d = nc.dram_tensor("Wv2", [HM, HM], bf16, kind="ExternalInput")
    bias_d = nc.dram_tensor("bias_row", [1, HCS], bf16, kind="ExternalInput")
    ones_d = nc.dram_tensor("ones_row", [1, P], f32r, kind="ExternalInput")
    onesb_d = nc.dram_tensor("ones_b", [1, P], bf16, kind="ExternalInput")
    out_d = nc.dram_tensor("pack_out", [E_LOC, OUT_W], bf16, kind="ExternalOutput")

    r32_view = r32_d.ap().rearrange("(n q p) w -> n p q w", q=Q, p=P)
    out_view = out_d.ap().rearrange("(n q p) w -> n p q w", q=Q, p=P)

    with tile.TileContext(nc) as tc, ExitStack() as ctx:
        consts = ctx.enter_context(tc.tile_pool(name="consts", bufs=1))
        io = ctx.enter_context(tc.tile_pool(name="io", bufs=3))
        prod = ctx.enter_context(tc.tile_pool(name="prod", bufs=3))
        asbp = ctx.enter_context(tc.tile_pool(name="asbp", bufs=3))
        outs = ctx.enter_context(tc.tile_pool(name="outs", bufs=3))
        rbp = ctx.enter_context(tc.tile_pool(name="rbp", bufs=1, space="PSUM"))
        pss = ctx.enter_context(tc.tile_pool(name="pss", bufs=2, space="PSUM"))
        psa = ctx.enter_context(tc.tile_pool(name="psa", bufs=1, space="PSUM"))
        psv = ctx.enter_context(tc.tile_pool(name="psv", bufs=2, space="PSUM"))

        ws1_t = consts.tile([HM, HCS], bf16)
        ws2_t = consts.tile([HM, HCS], bf16)
        wv1_t = consts.tile([HM, HM], bf16)
        wv2_t = consts.tile([HM, HM], bf16)
        bias_t = consts.tile([1, HCS], bf16)
        ones_t = consts.tile([1, P], f32r)
        onesb_t = consts.tile([1, P], bf16)
        nc.sync.dma_start(out=ws1_t[:], in_=ws1_d[:])
        nc.sync.dma_start(out=ws2_t[:], in_=ws2_d[:])
        nc.sync.dma_start(out=wv1_t[:], in_=wv1_d[:])
        nc.sync.dma_start(out=wv2_t[:], in_=wv2_d[:])
        nc.sync.dma_start(out=bias_t[:], in_=bias_d[:])
        nc.sync.dma_start(out=ones_t[:], in_=ones_d[:])
        nc.sync.dma_start(out=onesb_t[:], in_=onesb_d[:])

        for n in range(NMACRO):
            in_t = io.tile([P, 8, EM], bf16)
            rrow_t = io.tile([1, 3, EM], f32r)
            r_t = io.tile([P, Q, R_W], f32)
            nc.sync.dma_start(out=in_t[:], in_=in_d[:, n])
            nc.sync.dma_start(out=rrow_t[:], in_=rcm_d[n][None])
            nc.sync.dma_start(out=r_t[:], in_=r32_view[n])
            out_t = outs.tile([P, Q, OUT_W], bf16)

            # broadcast r across partitions: rb[p, k, e] = r[e, k]
            # (exact K=1 matmul; float32r runs at 1 cyc/row for N>=256)
            rb_t = rbp.tile([P, 3, EM], f32, tag="rb")
            for k in range(3):
                nc.tensor.matmul(
                    rb_t[:, k], ones_t[:], rrow_t[:, k],
                    start=True, stop=True,
                )

            w_ss = in_t[:, ROW_WSS]
            w_sv = in_t[:, ROW_WSV]
            w_vs = in_t[:, ROW_WVS]
            w_vv = in_t[:, ROW_WVV]
            s_cm = in_t[:, ROW_S]

            # channel-major products, one DVE op per 512 edges
            ss_t = prod.tile([P, EM], bf16, tag="ss")
            t_t = prod.tile([P, EM], bf16, tag="t")
            u_t = prod.tile([P, 3, EM], bf16, tag="u")
            d_t = prod.tile([P, 3, EM], f32, tag="d")
            e1_t = prod.tile([P, EM], f32, tag="e1")
            dot_t = prod.tile([P, EM], f32, tag="dot")
            vv_t = prod.tile([P, EM], bf16, tag="vv")
            nc.vector.tensor_tensor(out=ss_t[:], in0=w_ss, in1=s_cm, op=mult)
            nc.vector.tensor_tensor(out=t_t[:], in0=w_sv, in1=s_cm, op=mult)
            for k in range(3):
                nc.vector.tensor_tensor(
                    out=u_t[:, k], in0=w_vs, in1=in_t[:, ROW_V0 + k], op=mult
                )
                nc.vector.tensor_tensor(
                    out=d_t[:, k], in0=in_t[:, ROW_V0 + k], in1=rb_t[:, k],
                    op=mult,
                )
            nc.vector.tensor_tensor(out=e1_t[:], in0=d_t[:, 0], in1=d_t[:, 1], op=add)
            nc.vector.tensor_tensor(out=dot_t[:], in0=e1_t[:], in1=d_t[:, 2], op=add)
            nc.vector.tensor_tensor(out=vv_t[:], in0=dot_t[:], in1=w_vv, op=mult)

            for q in range(Q):
                sl = slice(q * P, (q + 1) * P)
                # block-diagonal matmuls, K=128=(h,m); outputs edge-major
                pss_t = pss.tile([P, HCS], f32, tag="pss")
                nc.tensor.matmul(
                    pss_t[:], ss_t[:, sl], ws1_t[:], start=True, stop=False
                )
                nc.tensor.matmul(
                    pss_t[:], vv_t[:, sl], ws2_t[:],
                    start=False, stop=not with_bias,
                )
                if with_bias:
                    nc.tensor.matmul(
                        pss_t[:], onesb_t[:], bias_t[:], start=False, stop=True
                    )
                psa_t = psa.tile([P, H, CV], f32, tag="psa")
                nc.tensor.matmul(
                    psa_t[:].rearrange("p h m -> p (h m)"), t_t[:, sl], wv1_t[:],
                    start=True, stop=True,
                )
                a_sb = asbp.tile([P, H, CV], f32, tag="a_sb")
                nc.scalar.copy(out=a_sb[:], in_=psa_t[:])
                psv_t = psv.tile([P, 3, H, CV], f32, tag="psv")
                for k in range(3):
                    nc.tensor.matmul(
                        psv_t[:, k].rearrange("p h m -> p (h m)"),
                        u_t[:, k, sl], wv2_t[:], start=True, stop=True,
                    )

                nc.scalar.copy(
                    out=out_t[:, q, SO_OFF : SO_OFF + HCS], in_=pss_t[:]
                )
                vo_b = out_t[:, q, VO_OFF : VO_OFF + HMK].rearrange(
                    "p (h m k) -> p h m k", h=H, k=3
                )
                for k in range(3):
                    nc.vector.scalar_tensor_tensor(
                        out=vo_b[:, :, :, k], in0=a_sb[:],
                        scalar=r_t[:, q, k : k + 1],
                        in1=psv_t[:, k], op0=mult, op1=add,
                    )
            nc.sync.dma_start(out=out_view[n], in_=out_t[:])

    nc.compile()
    return nc


def _prep_weights(w_s, b_s, w_v):
    bf16 = ml_dtypes.bfloat16
    ws1 = np.zeros((HM, HCS), np.float32)
    ws2 = np.zeros((HM, HCS), np.float32)
    wv1 = np.zeros((HM, HM), np.float32)
    wv2 = np.zeros((HM, HM), np.float32)
    for h in range(H):
        ws1[h * M : (h + 1) * M, h * CS : (h + 1) * CS] = w_s[h, :, 0:M].T
        ws2[h * M : (h + 1) * M, h * CS : (h + 1) * CS] = w_s[h, :, M : 2 * M].T
        wv1[h * M : (h + 1) * M, h * CV : (h + 1) * CV] = w_v[h, :, 0:M].T
        wv2[h * M : (h + 1) * M, h * CV : (h + 1) * CV] = w_v[h, :, M : 2 * M].T
    bias = np.asarray(b_s, np.float32).reshape(1, HCS)
    ones = np.ones((1, P), np.float32)
    return (
        ws1.astype(bf16), ws2.astype(bf16), wv1.astype(bf16), wv2.astype(bf16),
        bias.astype(bf16), ones, ones.astype(bf16),
    )


def _pack_core(s, v, w, lo, hi):
    """Channel-major bf16 pack: [hm=128, NMACRO, 8, EM]."""
    bf16 = ml_dtypes.bfloat16
    E = E_LOC
    pk = np.empty((P, NMACRO, 8, EM), bf16)
    wE = w[lo:hi].reshape(E, 4, HM)  # [e, c, hm]
    for c_idx in range(4):
        pk[:, :, c_idx, :] = wE[:, c_idx, :].T.reshape(HM, NMACRO, EM)
    s_cm = s[:, lo:hi].reshape(H, E, M).transpose(0, 2, 1).reshape(HM, E)
    pk[:, :, ROW_S, :] = s_cm.reshape(HM, NMACRO, EM)
    v_cm = v[:, lo:hi].reshape(H, E, M, 3).transpose(0, 2, 3, 1).reshape(HM, 3, E)
    for k in range(3):
        pk[:, :, ROW_V0 + k, :] = v_cm[:, k].reshape(HM, NMACRO, EM)
    return pk


def kernel(s, v, r_ij_vec, weights, w_s, b_s, w_v):
    from concourse.bass_utils import run_bass_kernel_spmd

    s = np.ascontiguousarray(np.asarray(s, np.float32))
    v = np.ascontiguousarray(np.asarray(v, np.float32))
    r = np.ascontiguousarray(np.asarray(r_ij_vec, np.float32))
    w = np.ascontiguousarray(np.asarray(weights, np.float32))
    b_s = np.asarray(b_s, np.float32)
    with_bias = bool(np.any(b_s != 0.0))
    nc = _build(with_bias)
    ws1, ws2, wv1, wv2, bias, ones, onesb = _prep_weights(
        np.asarray(w_s, np.float32), b_s, np.asarray(w_v, np.float32)
    )

    in_maps = []
    for c in range(NCORES):
        lo, hi = c * ILOC, (c + 1) * ILOC
        r_loc = r[lo:hi].reshape(E_LOC, 3)
        r_cm = np.ascontiguousarray(
            r_loc.reshape(NMACRO, EM, 3).transpose(0, 2, 1)
        )
        r32 = np.zeros((E_LOC, R_W), np.float32)
        r32[:, 0:3] = r_loc
        in_maps.append({
            "pack_cm": _pack_core(s, v, w, lo, hi),
            "r_cm": r_cm,
            "r32": r32,
            "Ws1": ws1, "Ws2": ws2, "Wv1": wv1, "Wv2": wv2,
            "bias_row": bias, "ones_row": ones, "ones_b": onesb,
        })

    res = run_bass_kernel_spmd(nc, in_maps, core_ids=list(range(NCORES)))

    s_parts, v_parts = [], []
    for c in range(NCORES):
        out = res.results[c]["pack_out"].astype(np.float32)
        s_parts.append(
            out[:, SO_OFF : SO_OFF + HCS]
            .reshape(ILOC, J, H, CS).transpose(2, 0, 1, 3)
        )
        v_parts.append(
            out[:, VO_OFF : VO_OFF + HMK]
            .reshape(ILOC, J, H, M, 3).transpose(2, 0, 1, 3, 4)
        )
    s_out = np.concatenate(s_parts, axis=1)
    v_out = np.concatenate(v_parts, axis=1)
    return s_out, v_out
